# revision 9
# baseline (speedup 1.0000x reference)
"""JointLoss Trainium2 kernel (v2 — engine-rebalanced).

Math (see reference):
  loss_pos[i] = ||f_i - agents[l_i]||^2                (host, exact fp32)
  neg[i]      = mean over masked j of relu(1 - dist[i,j])
  dist[i,j]   = f2[i] + a2[j] - 2 F@A.T
  answer      = (sum loss_pos + sum neg_src + sum neg_tgt) / (B + n_valid)

v1 (baseline) was DVE-bound: two 4000-wide tensor_reduce per block run at
1 elem/cycle (280us of 496us total).  v2 moves the row-sums off the DVE:

Device strategy (per core, 2048 rows, data-parallel over B):
  DMA  : sim block [128, 4000] f32 (5.7us — the roofline)
  DVE  : r = (sim > 0.5) {0,1} bf16;  w = min(h, r) (= masked hinge, h<=1)
  PE   : v = 2*F@A.T - a2 (bf16 + K=1 rank-1) -> PSUM  [8 x 512-chunks]
  ACT  : h = relu(v + (1 - f2)[i]) PSUM->SBUF bf16     [4 x 1024-chunks]
  PE   : row-sums as identity-matmul accumulation into PSUM:
           pw[i,jj] = sum_c w[i, 512c+jj]   (8 chunks)
           pr[i,jj] = sum_c rf[i, 512c+jj]  (4 chunks of Pool-folded r)
  Pool : rf = r[:, :2000] + r[:, 2000:]  (gpsimd fold halves the r-sum)
  DVE  : 512-wide PSUM reduces -> sw, cnt columns (0.66us each)
  label term removed per-row via host-precomputed correction columns;
  per-core partials [term_sum, n_valid] combined on host.
(tensor_tensor_reduce and activation accum_out crash this runtime's HW path —
 verified by isolation probes — hence the matmul-based reduction.)
"""

import os
from contextlib import ExitStack

import numpy as np

B, C, D = 16384, 4000, 128
NCORES = 8
BS = B // NCORES  # 2048 rows per core
NIB = BS // 128  # 16 row blocks per core
NSTREAM = 2  # src, tgt
CF = 2000  # fold width for the r-sum
NAB = C // 128  # 31 full agent blocks (last partial block handled separately)
CREM = C - NAB * 128  # 32
FOLD1_POOL = True  # r fold on gpsimd (fallback: DVE)
FP8_DIST = True  # dist matmul in fp8e4m3 DoubleRow (0.5 cyc/col) w/ K=65 affine row
XD = 3072  # mask-split: cols [0,XD) is_gt on DVE, [XD,C) sigmoid on ACT
BIGM = float(2**24)  # sigmoid saturation scale

_CACHE = {}


def _build_nc():
    import concourse.bacc as bacc
    import concourse.tile as tile
    from concourse import mybir
    from concourse.masks import make_identity

    f32 = mybir.dt.float32
    bf16 = mybir.dt.bfloat16
    Alu = mybir.AluOpType
    Act = mybir.ActivationFunctionType
    X = mybir.AxisListType.X

    nc = bacc.Bacc(
        "TRN2",
        target_bir_lowering=False,
        debug=False,
        enable_asserts=False,
        num_devices=NCORES,
    )

    f_d = nc.dram_tensor("f", (BS, D), f32, kind="ExternalInput").ap()
    ft_d = nc.dram_tensor("ft", (BS, D), f32, kind="ExternalInput").ap()
    ag_d = nc.dram_tensor("ag", (C, D), f32, kind="ExternalInput").ap()
    sim_d = nc.dram_tensor("sim", (BS, C), f32, kind="ExternalInput").ap()
    simt_d = nc.dram_tensor("simt", (BS, C), f32, kind="ExternalInput").ap()
    # host-precomputed per-row columns, packed partition-major [128, NIB]
    lp_d = nc.dram_tensor("lp", (128, NIB), f32, kind="ExternalInput").ap()
    corr_d = nc.dram_tensor("corr", (128, NIB), f32, kind="ExternalInput").ap()
    ml_d = nc.dram_tensor("ml", (128, NIB), f32, kind="ExternalInput").ap()
    out_d = nc.dram_tensor("out", (1, 2), f32, kind="ExternalOutput").ap()

    with tile.TileContext(nc) as tc, ExitStack() as ctx:
        const = ctx.enter_context(tc.tile_pool(name="const", bufs=1))
        setup = ctx.enter_context(tc.tile_pool(name="setup", bufs=2))
        work = ctx.enter_context(tc.tile_pool(name="work", bufs=2))
        deep = ctx.enter_context(tc.tile_pool(name="deep", bufs=3))
        psum = ctx.enter_context(tc.tile_pool(name="psum", bufs=2, space="PSUM"))

        ident = const.tile([128, 128], f32)
        make_identity(nc, ident)
        identb = const.tile([128, 128], bf16)
        nc.vector.tensor_scalar(identb, ident, 1.0, None, Alu.mult)
        ones_col = const.tile([128, 1], f32)
        nc.vector.memset(ones_col, 1.0)
        ones_row_bf = const.tile([1, 128], bf16)
        nc.vector.memset(ones_row_bf, 1.0)

        # persistent per-core state
        agT2 = const.tile([128, C], bf16)  # 2 * A.T
        nega2 = const.tile([1, C], bf16)  # -a2 row
        ftT = const.tile([128, NSTREAM * BS], bf16)  # [F.T | FT.T]
        bias_st = const.tile([128, NSTREAM * NIB], f32)  # 1 - f2
        cnt_st = const.tile([128, NSTREAM * NIB], f32)  # mask counts
        sw_st = const.tile([128, NSTREAM * NIB], f32)  # hinge sums
        lp_col = const.tile([128, NIB], f32)  # loss_pos (host)
        corr_col = const.tile([128, NIB], f32)  # label hinge corr (host)
        ml_col = const.tile([128, NIB], f32)  # (sim[i,l] > .5) (host)

        nc.sync.dma_start(out=lp_col, in_=lp_d)
        nc.sync.dma_start(out=corr_col, in_=corr_d)
        nc.sync.dma_start(out=ml_col, in_=ml_d)

        # ---- batched loads: agents (31 full blocks + 32-row tail), features ----
        abig = setup.tile([128, NAB * 128], f32, tag="abig", bufs=1)
        nc.sync.dma_start(
            out=abig.rearrange("p (b d) -> p b d", d=128),
            in_=ag_d[: NAB * 128, :].rearrange("(b p) d -> p b d", p=128),
        )
        atail = setup.tile([128, 128], f32, tag="atail", bufs=1)
        nc.vector.memset(atail, 0.0)
        nc.sync.dma_start(out=atail[:CREM, :], in_=ag_d[NAB * 128 :, :])
        fbig = {}
        for s, src in enumerate([f_d, ft_d]):
            fb = setup.tile([128, BS], f32, tag=f"fbig{s}", bufs=1)
            nc.sync.dma_start(
                out=fb.rearrange("p (b d) -> p b d", d=128),
                in_=src.rearrange("(b p) d -> p b d", p=128),
            )
            fbig[s] = fb

        # agents: transpose (batched x8 per PSUM tile), 2x scale, sq-norm row
        for grp in range(4):
            g0 = grp * 1024
            gw = min(1024, C - g0)
            pv = psum.tile([128, 1024], f32, tag="pv")
            for b in range(8):
                jb = grp * 8 + b
                if jb * 128 >= C:
                    break
                src_blk = (
                    abig[:, jb * 128 : (jb + 1) * 128] if jb < NAB else atail
                )
                nc.tensor.transpose(pv[:, b * 128 : (b + 1) * 128], src_blk, ident)
            nc.scalar.activation(
                out=agT2[:, g0 : g0 + gw], in_=pv[:, :gw], func=Act.Copy, scale=2.0
            )
            agsq = setup.tile([128, 1024], f32, tag="agsq")
            nc.scalar.activation(out=agsq[:, :gw], in_=pv[:, :gw], func=Act.Square)
            for c in (0, 512):
                n = min(512, gw - c)
                if n <= 0:
                    break
                pr = psum.tile([128, 512], f32, tag="pr")
                nc.tensor.matmul(
                    pr[0:1, :n],
                    lhsT=ones_col,
                    rhs=agsq[:, c : c + n],
                    start=True,
                    stop=True,
                )
                nc.scalar.activation(
                    out=nega2[0:1, g0 + c : g0 + c + n],
                    in_=pr[0:1, :n],
                    func=Act.Copy,
                    scale=-1.0,
                )

        # features: transpose batched, f2 per block on DVE
        for s in range(NSTREAM):
            for grp in range(2):
                pv = psum.tile([128, 1024], f32, tag="pv")
                for b in range(8):
                    ib = grp * 8 + b
                    fblk = fbig[s][:, ib * 128 : (ib + 1) * 128]
                    nc.tensor.transpose(pv[:, b * 128 : (b + 1) * 128], fblk, ident)
                    scr = setup.tile([128, D], f32, tag="fscr")
                    nc.vector.tensor_tensor(out=scr, in0=fblk, in1=fblk, op=Alu.mult)
                    sc = s * NIB + ib
                    nc.vector.tensor_reduce(
                        bias_st[:, sc : sc + 1], scr, axis=X, op=Alu.add
                    )
                col = s * BS + grp * 1024
                nc.scalar.activation(
                    out=ftT[:, col : col + 1024], in_=pv, func=Act.Copy
                )
        # bias = 1 - f2 (in place over the f2 accumulators)
        nc.scalar.activation(
            out=bias_st, in_=bias_st, func=Act.Copy, scale=-1.0, bias=1.0
        )

        # ---- main loop ----
        for s, simsrc in enumerate([sim_d, simt_d]):
            for ib in range(NIB):
                sc = s * NIB + ib
                sim_t = deep.tile([128, C], f32, tag="sim")
                nc.sync.dma_start(
                    out=sim_t, in_=simsrc[ib * 128 : (ib + 1) * 128, :]
                )
                r_t = deep.tile([128, C], bf16, tag="r")
                nc.vector.tensor_scalar(r_t, sim_t, 0.5, None, Alu.is_gt)
                rf_t = work.tile([128, CF], bf16, tag="rf")
                if FOLD1_POOL:
                    nc.gpsimd.tensor_tensor(
                        out=rf_t, in0=r_t[:, :CF], in1=r_t[:, CF:], op=Alu.add
                    )
                else:
                    nc.vector.tensor_tensor(
                        out=rf_t, in0=r_t[:, :CF], in1=r_t[:, CF:], op=Alu.add
                    )
                # r-sum on the folded mask
                pr = psum.tile([128, 512], f32, tag="pr")
                for c4 in range(4):
                    n = min(512, CF - c4 * 512)
                    nc.tensor.matmul(
                        pr[:, :n],
                        lhsT=identb,
                        rhs=rf_t[:, c4 * 512 : c4 * 512 + n],
                        start=(c4 == 0),
                        stop=(c4 == 3),
                    )
                h_t = work.tile([128, C], bf16, tag="h")
                w_t = work.tile([128, C], bf16, tag="w")
                pw = psum.tile([128, 512], f32, tag="pw")
                lhs = ftT[:, s * BS + ib * 128 : s * BS + (ib + 1) * 128]
                # chunked dist -> relu -> min -> w-sum pipeline: the w-sum
                # matmuls only wait on their own chunk's min, so the in-order
                # PE queue never stalls on the full-width mask apply
                for pc in range(4):
                    p0 = pc * 1024
                    pw_ = min(1024, C - p0)
                    pv = psum.tile([128, 1024], f32, tag="pv")
                    for c in (0, 512):
                        n = min(512, pw_ - c)
                        if n <= 0:
                            break
                        j0 = p0 + c
                        nc.tensor.matmul(
                            pv[:, c : c + n],
                            lhsT=lhs,
                            rhs=agT2[:, j0 : j0 + n],
                            start=True,
                            stop=False,
                        )
                        nc.tensor.matmul(
                            pv[:, c : c + n],
                            lhsT=ones_row_bf,
                            rhs=nega2[0:1, j0 : j0 + n],
                            start=False,
                            stop=True,
                        )
                    nc.scalar.activation(
                        out=h_t[:, p0 : p0 + pw_],
                        in_=pv[:, :pw_],
                        func=Act.Relu,
                        bias=bias_st[:, sc : sc + 1],
                    )
                    nc.vector.tensor_tensor(
                        out=w_t[:, p0 : p0 + pw_],
                        in0=h_t[:, p0 : p0 + pw_],
                        in1=r_t[:, p0 : p0 + pw_],
                        op=Alu.min,
                    )
                    for c in (0, 512):
                        n = min(512, pw_ - c)
                        if n <= 0:
                            break
                        nc.tensor.matmul(
                            pw[:, :n],
                            lhsT=identb,
                            rhs=w_t[:, p0 + c : p0 + c + n],
                            start=(pc == 0 and c == 0),
                            stop=(pc == 3 and c == 512),
                            skip_group_check=True,
                        )
                nc.vector.tensor_reduce(
                    sw_st[:, sc : sc + 1], pw, axis=X, op=Alu.add
                )
                nc.vector.tensor_reduce(
                    cnt_st[:, sc : sc + 1], pr, axis=X, op=Alu.add
                )

        # ---- finalize ----
        fin = ctx.enter_context(tc.tile_pool(name="fin", bufs=1))
        # src label corrections (host-precomputed columns)
        nc.vector.tensor_tensor(
            out=sw_st[:, :NIB], in0=sw_st[:, :NIB], in1=corr_col, op=Alu.subtract
        )
        nc.vector.tensor_tensor(
            out=cnt_st[:, :NIB], in0=cnt_st[:, :NIB], in1=ml_col, op=Alu.subtract
        )
        # neg = sw / max(cnt, 1); valid = cnt > 0
        den = fin.tile([128, NSTREAM * NIB], f32)
        nc.vector.tensor_scalar(den, cnt_st, 1.0, None, Alu.max)
        rec = fin.tile([128, NSTREAM * NIB], f32)
        nc.vector.reciprocal(rec, den)
        neg = fin.tile([128, NSTREAM * NIB], f32)
        nc.vector.tensor_tensor(out=neg, in0=sw_st, in1=rec, op=Alu.mult)
        valid = fin.tile([128, NSTREAM * NIB], f32)
        nc.vector.tensor_scalar(valid, cnt_st, 0.0, None, Alu.is_gt)
        # row totals
        tcol = fin.tile([128, 1], f32)
        t2 = fin.tile([128, 1], f32)
        nc.vector.tensor_reduce(tcol, neg, axis=X, op=Alu.add)
        nc.vector.tensor_reduce(t2, lp_col, axis=X, op=Alu.add)
        pack = fin.tile([128, 2], f32)
        nc.vector.tensor_tensor(out=pack[:, 0:1], in0=tcol, in1=t2, op=Alu.add)
        nc.vector.tensor_reduce(pack[:, 1:2], valid, axis=X, op=Alu.add)
        psf = psum.tile([128, 512], f32, tag="pr")
        nc.tensor.matmul(psf[0:1, 0:2], lhsT=ones_col, rhs=pack, start=True, stop=True)
        outt = fin.tile([1, 2], f32)
        nc.scalar.activation(out=outt, in_=psf[0:1, 0:2], func=Act.Copy)
        nc.sync.dma_start(out=out_d, in_=outt)

    nc.compile()
    return nc


def _get_nc():
    if "nc" not in _CACHE:
        _CACHE["nc"] = _build_nc()
    return _CACHE["nc"]


def _col128(x):
    # (BS,) row-major -> [128, NIB] partition-major blocks (i = ib*128 + p)
    return np.ascontiguousarray(x.reshape(NIB, 128).T, dtype=np.float32)


def make_in_maps(features, agents, labels, similarity, features_target, similarity_target):
    labels = np.asarray(labels).astype(np.int64)
    feats = np.asarray(features, dtype=np.float32)
    ags = np.asarray(agents, dtype=np.float32)
    diff = feats - ags[labels]
    lp_full = np.einsum("bd,bd->b", diff, diff).astype(np.float32)
    slbl_full = np.asarray(similarity)[np.arange(B), labels].astype(np.float32)
    ml_full = (slbl_full > 0.5).astype(np.float32)
    corr_full = (np.maximum(0.0, 1.0 - lp_full) * ml_full).astype(np.float32)
    c32 = lambda x: np.ascontiguousarray(x, dtype=np.float32)
    in_maps = []
    for c in range(NCORES):
        r = slice(c * BS, (c + 1) * BS)
        in_maps.append(
            {
                "f": c32(features[r]),
                "ft": c32(features_target[r]),
                "ag": c32(agents),
                "sim": c32(similarity[r]),
                "simt": c32(similarity_target[r]),
                "lp": _col128(lp_full[r]),
                "corr": _col128(corr_full[r]),
                "ml": _col128(ml_full[r]),
            }
        )
    return in_maps


def kernel(features, agents, labels, similarity, features_target, similarity_target):
    from concourse import bass_utils

    nc = _get_nc()
    in_maps = make_in_maps(
        features, agents, labels, similarity, features_target, similarity_target
    )
    res = bass_utils.run_bass_kernel_spmd(
        nc, in_maps, core_ids=list(range(NCORES)), trace=False
    )
    _CACHE["last_results"] = res
    parts = np.stack([r["out"][0] for r in res.results])  # [8, 2]
    term_sum = float(parts[:, 0].sum())
    n_valid = float(parts[:, 1].sum())
    return np.float32(term_sum / (B + n_valid))


# revision 27
# speedup vs baseline: 1.5092x; 1.5092x over previous
"""JointLoss Trainium2 kernel (v2 — engine-rebalanced).

Math (see reference):
  loss_pos[i] = ||f_i - agents[l_i]||^2                (host, exact fp32)
  neg[i]      = mean over masked j of relu(1 - dist[i,j])
  dist[i,j]   = f2[i] + a2[j] - 2 F@A.T
  answer      = (sum loss_pos + sum neg_src + sum neg_tgt) / (B + n_valid)

v1 (baseline) was DVE-bound: two 4000-wide tensor_reduce per block run at
1 elem/cycle (280us of 496us total).  v2 moves the row-sums off the DVE:

Device strategy (per core, 2048 rows, data-parallel over B):
  DMA  : sim block [128, 4000] f32 (5.7us — the roofline)
  DVE  : r = (sim > 0.5) {0,1} bf16;  w = min(h, r) (= masked hinge, h<=1)
  PE   : v = 2*F@A.T - a2 (bf16 + K=1 rank-1) -> PSUM  [8 x 512-chunks]
  ACT  : h = relu(v + (1 - f2)[i]) PSUM->SBUF bf16     [4 x 1024-chunks]
  PE   : row-sums as identity-matmul accumulation into PSUM:
           pw[i,jj] = sum_c w[i, 512c+jj]   (8 chunks)
           pr[i,jj] = sum_c rf[i, 512c+jj]  (4 chunks of Pool-folded r)
  Pool : rf = r[:, :2000] + r[:, 2000:]  (gpsimd fold halves the r-sum)
  DVE  : 512-wide PSUM reduces -> sw, cnt columns (0.66us each)
  label term removed per-row via host-precomputed correction columns;
  per-core partials [term_sum, n_valid] combined on host.
(tensor_tensor_reduce and activation accum_out crash this runtime's HW path —
 verified by isolation probes — hence the matmul-based reduction.)
"""

import os
from contextlib import ExitStack

import numpy as np

B, C, D = 16384, 4000, 128
NCORES = 8
BS = B // NCORES  # 2048 rows per core
NIB = BS // 128  # 16 row blocks per core
NSTREAM = 2  # src, tgt
CF = 2000  # fold width for the r-sum
NAB = C // 128  # 31 full agent blocks (last partial block handled separately)
CREM = C - NAB * 128  # 32
FOLD1_POOL = True  # r fold on gpsimd (fallback: DVE)
FP8_DIST = True  # dist matmul in fp8e4m3 DoubleRow (0.5 cyc/col) w/ K=65 affine row
XD = C  # mask-split point: cols [0,XD) is_gt on DVE, [XD,C) sigmoid on ACT
BIGM = float(2**24)  # sigmoid saturation scale
PSW = 256  # accumulation width of the w/r sum PSUMs

_CACHE = {}


def _build_nc():
    import concourse.bacc as bacc
    import concourse.tile as tile
    from concourse import mybir
    from concourse.masks import make_identity

    f32 = mybir.dt.float32
    bf16 = mybir.dt.bfloat16
    fp8 = mybir.dt.float8e4
    Alu = mybir.AluOpType
    Act = mybir.ActivationFunctionType
    X = mybir.AxisListType.X
    DR = mybir.MatmulPerfMode.DoubleRow

    nc = bacc.Bacc(
        "TRN2",
        target_bir_lowering=False,
        debug=False,
        enable_asserts=False,
        num_devices=NCORES,
    )

    f_d = nc.dram_tensor("f", (BS, D), f32, kind="ExternalInput").ap()
    ft_d = nc.dram_tensor("ft", (BS, D), f32, kind="ExternalInput").ap()
    ag_d = nc.dram_tensor("ag", (C, D), f32, kind="ExternalInput").ap()
    sim_d = nc.dram_tensor("sim", (BS, C), f32, kind="ExternalInput").ap()
    simt_d = nc.dram_tensor("simt", (BS, C), f32, kind="ExternalInput").ap()
    # host-precomputed per-row columns, packed partition-major [128, NIB]
    lp_d = nc.dram_tensor("lp", (128, NIB), f32, kind="ExternalInput").ap()
    corr_d = nc.dram_tensor("corr", (128, NIB), f32, kind="ExternalInput").ap()
    ml_d = nc.dram_tensor("ml", (128, NIB), f32, kind="ExternalInput").ap()
    out_d = nc.dram_tensor("out", (1, 2), f32, kind="ExternalOutput").ap()

    with tile.TileContext(nc) as tc, ExitStack() as ctx:
        const = ctx.enter_context(tc.tile_pool(name="const", bufs=1))
        setup = ctx.enter_context(tc.tile_pool(name="setup", bufs=2))
        work = ctx.enter_context(tc.tile_pool(name="work", bufs=2))
        deep = ctx.enter_context(tc.tile_pool(name="deep", bufs=3))
        rpool = ctx.enter_context(tc.tile_pool(name="rpool", bufs=4))
        psum = ctx.enter_context(tc.tile_pool(name="psum", bufs=2, space="PSUM"))

        ident = const.tile([128, 128], f32)
        make_identity(nc, ident)
        identb = const.tile([128, 128], bf16)
        nc.vector.tensor_scalar(identb, ident, 1.0, None, Alu.mult)
        ones_col = const.tile([128, 1], f32)
        nc.vector.memset(ones_col, 1.0)
        ones_row_bf = const.tile([1, 128], bf16)
        nc.vector.memset(ones_row_bf, 1.0)

        # persistent per-core state
        agT2 = const.tile([128, C], bf16)  # 2 * A.T
        nega2 = const.tile([1, C], bf16)  # -a2 row
        ftT = const.tile([128, NSTREAM * BS], bf16)  # [F.T | FT.T]
        if FP8_DIST:
            # DoubleRow fp8 operands: [65, 2, *] with d-halves on the k-subtile
            # axis and row 64 carrying the (ones x -a2) affine term
            agT28 = const.tile([65, 2 * C], fp8)
            ftT8 = const.tile([65, 2 * NSTREAM * BS], fp8)
            row65 = const.tile([65, C], bf16)  # partition-hop scratch
            bigneg = const.tile([128, 1], f32)
            nc.vector.memset(bigneg, -0.5 * BIGM)
        bias_st = const.tile([128, NSTREAM * NIB], f32)  # 1 - f2
        cnt_st = const.tile([128, NSTREAM * NIB], f32)  # mask counts
        sw_st = const.tile([128, NSTREAM * NIB], f32)  # hinge sums
        lp_col = const.tile([128, NIB], f32)  # loss_pos (host)
        corr_col = const.tile([128, NIB], f32)  # label hinge corr (host)
        ml_col = const.tile([128, NIB], f32)  # (sim[i,l] > .5) (host)

        nc.sync.dma_start(out=lp_col, in_=lp_d)
        nc.sync.dma_start(out=corr_col, in_=corr_d)
        nc.sync.dma_start(out=ml_col, in_=ml_d)

        # ---- batched loads: agents (31 full blocks + 32-row tail), features ----
        abig = setup.tile([128, NAB * 128], f32, tag="abig", bufs=1)
        nc.sync.dma_start(
            out=abig.rearrange("p (b d) -> p b d", d=128),
            in_=ag_d[: NAB * 128, :].rearrange("(b p) d -> p b d", p=128),
        )
        atail = setup.tile([128, 128], f32, tag="atail", bufs=1)
        nc.vector.memset(atail, 0.0)
        nc.sync.dma_start(out=atail[:CREM, :], in_=ag_d[NAB * 128 :, :])
        fbig = {}
        for s, src in enumerate([f_d, ft_d]):
            fb = setup.tile([128, BS], f32, tag="fbig", bufs=2)
            nc.sync.dma_start(
                out=fb.rearrange("p (b d) -> p b d", d=128),
                in_=src.rearrange("(b p) d -> p b d", p=128),
            )
            fbig[s] = fb

        # agents: transpose (batched x8 per PSUM tile), 2x scale, sq-norm row
        for grp in range(4):
            g0 = grp * 1024
            gw = min(1024, C - g0)
            pv = psum.tile([128, 1024], f32, tag="pv")
            for b in range(8):
                jb = grp * 8 + b
                if jb * 128 >= C:
                    break
                src_blk = (
                    abig[:, jb * 128 : (jb + 1) * 128] if jb < NAB else atail
                )
                nc.tensor.transpose(pv[:, b * 128 : (b + 1) * 128], src_blk, ident)
            nc.scalar.activation(
                out=agT2[:, g0 : g0 + gw], in_=pv[:, :gw], func=Act.Copy, scale=2.0
            )
            agsq = setup.tile([128, 1024], f32, tag="agsq")
            nc.scalar.activation(out=agsq[:, :gw], in_=pv[:, :gw], func=Act.Square)
            for c in (0, 512):
                n = min(512, gw - c)
                if n <= 0:
                    break
                pr = psum.tile([128, 512], f32, tag="pr")
                nc.tensor.matmul(
                    pr[0:1, :n],
                    lhsT=ones_col,
                    rhs=agsq[:, c : c + n],
                    start=True,
                    stop=True,
                )
                nc.scalar.activation(
                    out=nega2[0:1, g0 + c : g0 + c + n],
                    in_=pr[0:1, :n],
                    func=Act.Copy,
                    scale=-1.0,
                )

        # features: transpose batched, f2 per block on DVE
        for s in range(NSTREAM):
            for grp in range(2):
                pv = psum.tile([128, 1024], f32, tag="pv")
                for b in range(8):
                    ib = grp * 8 + b
                    fblk = fbig[s][:, ib * 128 : (ib + 1) * 128]
                    nc.tensor.transpose(pv[:, b * 128 : (b + 1) * 128], fblk, ident)
                    scr = setup.tile([128, D], f32, tag="fscr")
                    nc.vector.tensor_tensor(out=scr, in0=fblk, in1=fblk, op=Alu.mult)
                    sc = s * NIB + ib
                    nc.vector.tensor_reduce(
                        bias_st[:, sc : sc + 1], scr, axis=X, op=Alu.add
                    )
                col = s * BS + grp * 1024
                nc.scalar.activation(
                    out=ftT[:, col : col + 1024], in_=pv, func=Act.Copy
                )
        # bias = 1 - f2 (in place over the f2 accumulators)
        nc.scalar.activation(
            out=bias_st, in_=bias_st, func=Act.Copy, scale=-1.0, bias=1.0
        )

        if FP8_DIST:
            # fp8 DoubleRow operand prep.  Half 0 (d 0..63) converts in place;
            # half 1 (d 64..127) must hop partitions via SBUF->SBUF DMA first.
            nc.scalar.activation(
                out=agT28[0:64, 0:C], in_=agT2[0:64, :], func=Act.Copy
            )
            nc.sync.dma_start(out=row65[0:64, :], in_=agT2[64:128, :])
            nc.scalar.activation(
                out=agT28[0:64, C : 2 * C], in_=row65[0:64, :], func=Act.Copy
            )
            # affine row: sub0 = -a2 (fp8), sub1 = 0
            nc.sync.dma_start(out=row65[64:65, :], in_=nega2)
            nc.scalar.activation(
                out=agT28[64:65, 0:C], in_=row65[64:65, :], func=Act.Copy, scale=0.5
            )
            nc.vector.memset(agT28[64:65, C : 2 * C], 0.0)
            for s in range(NSTREAM):
                f0 = s * BS
                o0 = s * 2 * BS
                nc.scalar.activation(
                    out=ftT8[0:64, o0 : o0 + BS],
                    in_=ftT[0:64, f0 : f0 + BS],
                    func=Act.Copy,
                )
                nc.sync.dma_start(
                    out=row65[0:64, :BS], in_=ftT[64:128, f0 : f0 + BS]
                )
                nc.scalar.activation(
                    out=ftT8[0:64, o0 + BS : o0 + 2 * BS],
                    in_=row65[0:64, :BS],
                    func=Act.Copy,
                )
                # ones row (pairs with -a2), zero on sub1
                nc.vector.memset(ftT8[64:65, o0 : o0 + BS], 2.0)
                nc.vector.memset(ftT8[64:65, o0 + BS : o0 + 2 * BS], 0.0)
            agv = agT28.rearrange("p (two n) -> p two n", two=2)
            ftv = ftT8.rearrange("p (st two m) -> p st two m", st=NSTREAM, two=2)

        # ---- main loop ----
        pend = None  # (pw, pr, sc): reduces deferred into the next iter's
        # DVE idle window so is_gt never queues behind them

        def emit_reds(p):
            pw_p, pr_p, sc_p = p
            nc.vector.tensor_reduce(
                sw_st[:, sc_p : sc_p + 1], pw_p, axis=X, op=Alu.add
            )
            nc.vector.tensor_reduce(
                cnt_st[:, sc_p : sc_p + 1], pr_p, axis=X, op=Alu.add
            )

        for s, simsrc in enumerate([sim_d, simt_d]):
            for ib in range(NIB):
                sc = s * NIB + ib
                sim_t = deep.tile([128, C], f32, tag="sim")
                nc.sync.dma_start(
                    out=sim_t, in_=simsrc[ib * 128 : (ib + 1) * 128, :]
                )
                r_t = rpool.tile([128, C], bf16, tag="r")
                if XD < C:
                    # mask split: DVE is_gt head, ACT saturated-sigmoid tail
                    nc.vector.tensor_scalar(
                        r_t[:, :XD], sim_t[:, :XD], 0.5, None, Alu.is_gt
                    )
                    nc.scalar.activation(
                        out=r_t[:, XD:],
                        in_=sim_t[:, XD:],
                        func=Act.Sigmoid,
                        scale=BIGM,
                        bias=bigneg,
                    )
                else:
                    nc.vector.tensor_scalar(r_t, sim_t, 0.5, None, Alu.is_gt)
                if pend is not None:
                    emit_reds(pend)
                # r-sum straight off the mask (identity-matmul accumulation)
                pr = psum.tile([128, PSW], f32, tag="pr")
                nrc = (C + PSW - 1) // PSW
                for c4 in range(nrc):
                    n = min(PSW, C - c4 * PSW)
                    nc.tensor.matmul(
                        pr[:, :n],
                        lhsT=identb,
                        rhs=r_t[:, c4 * PSW : c4 * PSW + n],
                        start=(c4 == 0),
                        stop=(c4 == nrc - 1),
                    )
                h_t = work.tile([128, C], bf16, tag="h")
                w_t = work.tile([128, C], bf16, tag="w")
                pw = psum.tile([128, PSW], f32, tag="pw")
                lhs = ftT[:, s * BS + ib * 128 : s * BS + (ib + 1) * 128]
                # chunked dist -> relu -> min -> w-sum pipeline: the w-sum
                # matmuls only wait on their own chunk's min, so the in-order
                # PE queue never stalls on the full-width mask apply
                for pc in range(4):
                    p0 = pc * 1024
                    pw_ = min(1024, C - p0)
                    pv = psum.tile([128, 1024], f32, tag="pv")
                    for c in (0, 512):
                        n = min(512, pw_ - c)
                        if n <= 0:
                            break
                        j0 = p0 + c
                        if FP8_DIST:
                            nc.tensor.matmul(
                                pv[:, c : c + n],
                                lhsT=ftv[:, s, :, ib * 128 : (ib + 1) * 128],
                                rhs=agv[:, :, j0 : j0 + n],
                                start=True,
                                stop=True,
                                perf_mode=DR,
                            )
                        else:
                            nc.tensor.matmul(
                                pv[:, c : c + n],
                                lhsT=lhs,
                                rhs=agT2[:, j0 : j0 + n],
                                start=True,
                                stop=False,
                            )
                            nc.tensor.matmul(
                                pv[:, c : c + n],
                                lhsT=ones_row_bf,
                                rhs=nega2[0:1, j0 : j0 + n],
                                start=False,
                                stop=True,
                            )
                    nc.scalar.activation(
                        out=h_t[:, p0 : p0 + pw_],
                        in_=pv[:, :pw_],
                        func=Act.Relu,
                        bias=bias_st[:, sc : sc + 1],
                    )
                    nc.vector.tensor_tensor(
                        out=w_t[:, p0 : p0 + pw_],
                        in0=h_t[:, p0 : p0 + pw_],
                        in1=r_t[:, p0 : p0 + pw_],
                        op=Alu.min,
                    )
                    for c in range(0, pw_, PSW):
                        n = min(PSW, pw_ - c)
                        nc.tensor.matmul(
                            pw[:, :n],
                            lhsT=identb,
                            rhs=w_t[:, p0 + c : p0 + c + n],
                            start=(pc == 0 and c == 0),
                            stop=(pc == 3 and c + n == pw_),
                            skip_group_check=True,
                        )
                pend = (pw, pr, sc)
        emit_reds(pend)

        # ---- finalize ----
        fin = ctx.enter_context(tc.tile_pool(name="fin", bufs=1))
        # src label corrections (host-precomputed columns)
        nc.vector.tensor_tensor(
            out=sw_st[:, :NIB], in0=sw_st[:, :NIB], in1=corr_col, op=Alu.subtract
        )
        nc.vector.tensor_tensor(
            out=cnt_st[:, :NIB], in0=cnt_st[:, :NIB], in1=ml_col, op=Alu.subtract
        )
        # neg = sw / max(cnt, 1); valid = cnt > 0
        den = fin.tile([128, NSTREAM * NIB], f32)
        nc.vector.tensor_scalar(den, cnt_st, 1.0, None, Alu.max)
        rec = fin.tile([128, NSTREAM * NIB], f32)
        nc.vector.reciprocal(rec, den)
        neg = fin.tile([128, NSTREAM * NIB], f32)
        nc.vector.tensor_tensor(out=neg, in0=sw_st, in1=rec, op=Alu.mult)
        valid = fin.tile([128, NSTREAM * NIB], f32)
        nc.vector.tensor_scalar(valid, cnt_st, 0.0, None, Alu.is_gt)
        # row totals
        tcol = fin.tile([128, 1], f32)
        t2 = fin.tile([128, 1], f32)
        nc.vector.tensor_reduce(tcol, neg, axis=X, op=Alu.add)
        nc.vector.tensor_reduce(t2, lp_col, axis=X, op=Alu.add)
        pack = fin.tile([128, 2], f32)
        nc.vector.tensor_tensor(out=pack[:, 0:1], in0=tcol, in1=t2, op=Alu.add)
        nc.vector.tensor_reduce(pack[:, 1:2], valid, axis=X, op=Alu.add)
        psf = psum.tile([128, 512], f32, tag="pr")
        nc.tensor.matmul(psf[0:1, 0:2], lhsT=ones_col, rhs=pack, start=True, stop=True)
        outt = fin.tile([1, 2], f32)
        nc.scalar.activation(out=outt, in_=psf[0:1, 0:2], func=Act.Copy)
        nc.sync.dma_start(out=out_d, in_=outt)

    nc.compile()
    return nc


def _get_nc():
    if "nc" not in _CACHE:
        _CACHE["nc"] = _build_nc()
    return _CACHE["nc"]


def _col128(x):
    # (BS,) row-major -> [128, NIB] partition-major blocks (i = ib*128 + p)
    return np.ascontiguousarray(x.reshape(NIB, 128).T, dtype=np.float32)


def make_in_maps(features, agents, labels, similarity, features_target, similarity_target):
    labels = np.asarray(labels).astype(np.int64)
    feats = np.asarray(features, dtype=np.float32)
    ags = np.asarray(agents, dtype=np.float32)
    diff = feats - ags[labels]
    lp_full = np.einsum("bd,bd->b", diff, diff).astype(np.float32)
    slbl_full = np.asarray(similarity)[np.arange(B), labels].astype(np.float32)
    ml_full = (slbl_full > 0.5).astype(np.float32)
    corr_full = (np.maximum(0.0, 1.0 - lp_full) * ml_full).astype(np.float32)
    c32 = lambda x: np.ascontiguousarray(x, dtype=np.float32)
    in_maps = []
    for c in range(NCORES):
        r = slice(c * BS, (c + 1) * BS)
        in_maps.append(
            {
                "f": c32(features[r]),
                "ft": c32(features_target[r]),
                "ag": c32(agents),
                "sim": c32(similarity[r]),
                "simt": c32(similarity_target[r]),
                "lp": _col128(lp_full[r]),
                "corr": _col128(corr_full[r]),
                "ml": _col128(ml_full[r]),
            }
        )
    return in_maps


def kernel(features, agents, labels, similarity, features_target, similarity_target):
    from concourse import bass_utils

    nc = _get_nc()
    in_maps = make_in_maps(
        features, agents, labels, similarity, features_target, similarity_target
    )
    res = bass_utils.run_bass_kernel_spmd(
        nc, in_maps, core_ids=list(range(NCORES)), trace=False
    )
    _CACHE["last_results"] = res
    parts = np.stack([r["out"][0] for r in res.results])  # [8, 2]
    term_sum = float(parts[:, 0].sum())
    n_valid = float(parts[:, 1].sum())
    return np.float32(term_sum / (B + n_valid))
